# revision 14
# baseline (speedup 1.0000x reference)
"""Multi-head causal self-attention (B=2, T=4096, D=768, H=12) on 8 trn2 cores.

Sharding: core c -> batch b = c//4, heads 3*(c%4) .. 3*(c%4)+2.
qkv_proj column-parallel (each core computes Q/K/V only for its heads),
out_proj row-parallel (each core emits a partial y^T; host sums the 4
partials per batch).

v3 (bf16, ACT-saturated): all matmul operands are bf16 (fp32 matmuls run
at 1/4 PE rate; bf16 at full rate), accumulation stays fp32 in PSUM.
x is pre-transposed on the host so the kernel needs no PE transposes.
The ScalarE exp stream is the bottleneck (~1us per [128,1024] score
tile, ~215us total), so everything else is scheduled to hide inside it:

- scores S^T = K Q^T per 128-row k-tile, heads 0/1 row-paired on
  opposite PE halves (concurrent via row groups) into a 2-bank PSUM
  tile; one exp covers both heads.  Head 2 runs in a second pass
  self-paired via a partition-swapped Q2/K2 copy.
- score matmuls are software-pipelined one k-tile ahead of the exp.
- AV accumulates per head with a ones-row appended to V so softmax
  denominators fall out as row 64.  Dedicated PSUM slots per head
  (tag "av", bufs=3) so head-2 accumulation never waits on the
  head-0/1 normalize.
- normalization: denominators are DMA-reshaped [1,512]->[128,4] so the
  DVE reciprocal runs across 128 lanes (a [1,512] reciprocal costs
  3.3us on one lane), then gpsimd partition_broadcast + DVE multiply.
- phase A (projections) for t-block 0 runs upfront; projections for
  block qb+1 and the out-projection of block qb-1 are drip-fed one
  chunk per k-tile into block qb's attention loop so the PE never
  starves the exp stream.
"""

import sys

sys.path.insert(0, "/opt/trn_rl_repo")

import numpy as np
from contextlib import ExitStack

import concourse.bass as bass
import concourse.bacc as bacc
import concourse.tile as tile
import concourse.mybir as mybir
from concourse.bass_utils import run_bass_kernel_spmd

F32 = mybir.dt.float32
BF16 = mybir.dt.bfloat16
AF = mybir.ActivationFunctionType

B = 2
T = 4096
D = 768
H = 12
DK = 64
NCORES = 8
HL = 3  # heads per core
ND = D // 128  # 6 d-tiles
NKT = T // 128  # 32 k-tiles
NQB = T // 512  # 8 q-blocks

_CACHE = {}


def _emit(tc):
    nc = tc.nc
    xT_d = nc.dram_tensor("xT", [D, T], BF16, kind="ExternalInput").ap()
    wqk_d = nc.dram_tensor("wqkT", [D, 8 * DK], BF16, kind="ExternalInput").ap()
    wv_d = nc.dram_tensor("wvT", [D, HL * DK], BF16, kind="ExternalInput").ap()
    wo01_d = nc.dram_tensor("wo01T", [128, D], BF16, kind="ExternalInput").ap()
    wo2_d = nc.dram_tensor("wo2T", [DK, D], BF16, kind="ExternalInput").ap()
    y_d = nc.dram_tensor("yT", [D, T], F32, kind="ExternalOutput").ap()

    ctx = ExitStack()
    const = ctx.enter_context(tc.tile_pool(name="const", bufs=1))
    persist = ctx.enter_context(tc.tile_pool(name="persist", bufs=1))
    xtpool = ctx.enter_context(tc.tile_pool(name="xt", bufs=2))
    ptpool = ctx.enter_context(tc.tile_pool(name="pt", bufs=6))
    spool = ctx.enter_context(tc.tile_pool(name="sp", bufs=6))
    # PSUM (8 banks): psS tag "ps" 2x[128,1024] = 4 banks (score tiles,
    # double-buffered for the exp stream); psAV tag "av" 3x[65,512] = 3
    # banks (one AV accumulator per head); tag "x" 1x[128,512] = 1 bank
    # (projection chunks + out-proj, strictly serialized drip-feed work).
    psS = ctx.enter_context(tc.tile_pool(name="psS", bufs=2, space="PSUM"))
    psAV = ctx.enter_context(tc.tile_pool(name="psAV", bufs=1, space="PSUM"))

    # ---- weights ----
    wqk_sb = const.tile([128, ND, 8 * DK], BF16)
    nc.sync.dma_start(out=wqk_sb, in_=wqk_d.rearrange("(j p) e -> p j e", p=128))
    wv_sb = const.tile([128, ND, HL * DK], BF16)
    nc.sync.dma_start(out=wv_sb, in_=wv_d.rearrange("(j p) e -> p j e", p=128))
    wo01_sb = const.tile([128, D], BF16)
    nc.sync.dma_start(out=wo01_sb, in_=wo01_d)
    wo2_sb = const.tile([DK, D], BF16)
    nc.sync.dma_start(out=wo2_sb, in_=wo2_d)

    # warm the exp table set while weights stream in
    warm_in = const.tile([1, 16], F32)
    nc.vector.memset(warm_in, 0.0)
    warm_out = const.tile([1, 16], F32)
    nc.scalar.activation(warm_out, warm_in, AF.Exp, scale=0.125)

    # causal band masks, [k, q]-layout: mask[bp][k, q] = (q >= 128*bp + k).
    # bandA[bp]: same mask in both 512-halves (heads 0/1, same k-tile).
    # bandB[j]:  halves are bp=2j and bp=2j+1 (head 2, k-tile pair).
    def band_fill(m, half, bp):
        nc.gpsimd.affine_select(
            out=m[:, half * 512 : (half + 1) * 512],
            in_=m[:, half * 512 : (half + 1) * 512],
            compare_op=mybir.AluOpType.is_ge, fill=0.0,
            base=-128 * bp, pattern=[[1, 512]], channel_multiplier=-1,
        )

    bandA = []
    for bp in range(4):
        m = const.tile([128, 1024], BF16, name=f"bandA{bp}")
        nc.gpsimd.memset(m, 1.0)
        band_fill(m, 0, bp)
        band_fill(m, 1, bp)
        bandA.append(m)
    bandB = []
    for j in range(2):
        m = const.tile([128, 1024], BF16, name=f"bandB{j}")
        nc.gpsimd.memset(m, 1.0)
        band_fill(m, 0, 2 * j)
        band_fill(m, 1, 2 * j + 1)
        bandB.append(m)

    # ---- persistent activations ----
    # KA: [K^T_h0 ; K^T_h1], QB: [Q^T_h0 ; Q^T_h1] on partition halves
    KA = persist.tile([128, T], BF16, name="KA")
    QB = persist.tile([128, T], BF16, name="QB")
    C2 = persist.tile([128, T], BF16, name="C2")  # [K^T_h2 ; Q^T_h2]
    D2 = persist.tile([128, T], BF16, name="D2")  # [Q^T_h2 ; K^T_h2] (swapped)
    # V natural [t, e] per k-tile with a ones col at e=64 -> softmax sums
    Vall = persist.tile([128, NKT, HL, DK + 1], BF16, name="Vall")
    nc.gpsimd.memset(Vall[:, :, :, DK : DK + 1], 1.0)
    ot01 = persist.tile([128, 512], BF16, name="ot01")  # heads 0/1 out^T
    ot2 = persist.tile([DK, 512], BF16, name="ot2")
    y_acc = persist.tile([128, ND, 512], F32, name="y_acc")  # out-proj staging

    qk_dest = [KA, QB, C2, D2]
    xts = {}

    def emit_xt_dma(tsb):
        tblk = slice(tsb * 512, (tsb + 1) * 512)
        xt_sb = xtpool.tile([128, ND, 512], BF16, name="xt_sb")
        nc.sync.dma_start(
            out=xt_sb, in_=xT_d[:, tblk].rearrange("(j p) t -> p j t", p=128)
        )
        xts[tsb] = xt_sb

    def proj_chunks(tsb):
        """PE chunk thunks for t-superblock tsb's projections (~0.6-1.3us
        of PE work each), drip-fed between attention k-tiles."""
        tblk = slice(tsb * 512, (tsb + 1) * 512)

        def c_q(et):
            # et=3 projects [Q2|K2] (swapped weight columns) straight into
            # D2 - no partition-swap DMA needed
            def thunk():
                xt_sb = xts[tsb]
                ps_q = psAV.tile([128, 512], F32, name="ps_q", tag="x")
                for dj in range(ND):
                    nc.tensor.matmul(
                        ps_q,
                        lhsT=wqk_sb[:, dj, et * 128 : (et + 1) * 128],
                        rhs=xt_sb[:, dj, :],
                        start=(dj == 0), stop=(dj == ND - 1),
                    )
                nc.vector.tensor_copy(qk_dest[et][:, tblk], ps_q)
            return thunk

        def c_v(tt):
            def thunk():
                xt_sb = xts[tsb]
                kt = tsb * 4 + tt
                ps_v = psAV.tile([128, HL * DK], F32, name="ps_v", tag="x")
                for dj in range(ND):
                    nc.tensor.matmul(
                        ps_v,
                        lhsT=xt_sb[:, dj, tt * 128 : (tt + 1) * 128],
                        rhs=wv_sb[:, dj, :],
                        start=(dj == 0), stop=(dj == ND - 1),
                    )
                nc.vector.tensor_copy(
                    Vall[:, kt, :, 0:DK],
                    ps_v.rearrange("p (h e) -> p h e", h=HL),
                )
            return thunk

        return [c_q(0), c_q(1), c_q(2), c_q(3), c_v(0), c_v(1), c_v(2), c_v(3)]

    # phase A for t-block 0 runs upfront (nothing to hide it behind)
    emit_xt_dma(0)
    if NQB > 1:
        emit_xt_dma(1)
    for thunk in proj_chunks(0):
        thunk()

    # ================= attention =================
    def normalize(av, dest):
        """dest[e, q] = av[e, q] / av[64, q] (softmax denominators).
        ~18-bit approximate reciprocal (5x faster than the exact op, and
        denominators are well-conditioned: sums of positives in
        [7e-3, 1e6])."""
        sums_sb = spool.tile([1, 512], F32, name="sums_sb")
        nc.vector.tensor_copy(sums_sb, av[DK : DK + 1, :])
        recip = spool.tile([1, 512], F32, name="recip")
        nc.vector.reciprocal_approx_fast(recip, sums_sb)
        recipb = spool.tile([DK, 512], F32, name="recipb")
        nc.gpsimd.partition_broadcast(recipb, recip, channels=DK)
        nc.vector.tensor_mul(dest, av[0:DK, :], recipb)

    pending = []  # drip-feed thunks: out-proj of qb-1, projections of qb+1

    for qb in range(NQB):
        nk = 4 * (qb + 1)
        qblk = slice(qb * 512, (qb + 1) * 512)
        if qb + 2 < NQB:
            emit_xt_dma(qb + 2)  # prefetch x^T for the block after next
        if qb + 1 < NQB:
            pending.extend(proj_chunks(qb + 1))

        # ---- pass 0: heads 0/1, row-paired on opposite PE halves ----
        av0 = psAV.tile([DK + 1, 512], F32, name="av0", tag="av", bufs=3)
        av1 = psAV.tile([DK + 1, 512], F32, name="av1", tag="av", bufs=3)
        avs = [av0, av1]

        def scores01(kt):
            pss = psS.tile([128, 1024], F32, name="pss", tag="ps")
            kblk = slice(kt * 128, (kt + 1) * 128)
            nc.tensor.matmul(
                pss[:, 0:512], lhsT=KA[0:64, kblk], rhs=QB[0:64, qblk],
                start=True, stop=True,
            )
            nc.tensor.matmul(
                pss[:, 512:1024], lhsT=KA[64:128, kblk], rhs=QB[64:128, qblk],
                start=True, stop=True,
            )
            return pss

        pss_next = scores01(0)
        for kt in range(nk):
            pss_cur = pss_next
            if kt + 1 < nk:
                pss_next = scores01(kt + 1)
            if pending:
                pending.pop(0)()
            pt = ptpool.tile([128, 1024], BF16, name="pt")
            nc.scalar.activation(pt, pss_cur, AF.Exp, scale=0.125)
            if kt >= 4 * qb:
                nc.vector.tensor_mul(pt, pt, bandA[kt - 4 * qb])
            for h in (0, 1):
                nc.tensor.matmul(
                    avs[h],
                    lhsT=Vall[:, kt, h, :], rhs=pt[:, h * 512 : (h + 1) * 512],
                    start=(kt == 0), stop=(kt == nk - 1),
                )

        # ---- pass 1: head 2, self-paired k-tile pairs via C2/D2 ----
        av2 = psAV.tile([DK + 1, 512], F32, name="av2", tag="av", bufs=3)

        def scores2(kp):
            pss = psS.tile([128, 1024], F32, name="pss2", tag="ps")
            b0 = slice((2 * kp) * 128, (2 * kp + 1) * 128)
            b1 = slice((2 * kp + 1) * 128, (2 * kp + 2) * 128)
            nc.tensor.matmul(
                pss[:, 0:512], lhsT=C2[0:64, b0], rhs=D2[0:64, qblk],
                start=True, stop=True,
            )
            nc.tensor.matmul(
                pss[:, 512:1024], lhsT=D2[64:128, b1], rhs=C2[64:128, qblk],
                start=True, stop=True,
            )
            return pss

        nkp = nk // 2
        pss_next = scores2(0)
        for kp in range(nkp):
            pss_cur = pss_next
            if kp + 1 < nkp:
                pss_next = scores2(kp + 1)
            if pending:
                pending.pop(0)()
            pt2 = ptpool.tile([128, 1024], BF16, name="pt2")
            nc.scalar.activation(pt2, pss_cur, AF.Exp, scale=0.125)
            if 2 * kp >= 4 * qb:
                nc.vector.tensor_mul(pt2, pt2, bandB[kp - 2 * qb])
            nc.tensor.matmul(
                av2, lhsT=Vall[:, 2 * kp, 2, :], rhs=pt2[:, 0:512],
                start=(kp == 0), stop=False,
            )
            nc.tensor.matmul(
                av2, lhsT=Vall[:, 2 * kp + 1, 2, :], rhs=pt2[:, 512:1024],
                start=False, stop=(kp == nkp - 1),
            )

        # flush leftovers so out-proj of qb-1 is done before ot01 rewrite
        while pending:
            pending.pop(0)()

        normalize(av0, ot01[0:DK, :])
        ot1s = spool.tile([DK, 512], BF16, name="ot1s")
        normalize(av1, ot1s)
        nc.sync.dma_start(out=ot01[DK:128, :], in_=ot1s)
        normalize(av2, ot2)

        # ---- out-proj: y^T[d, q] = Wo01^T.T ot01 + Wo2^T.T ot2 ----
        def make_outproj(dj, qblk=qblk):
            def thunk():
                psy = psAV.tile([128, 512], F32, name="psy", tag="x")
                nc.tensor.matmul(
                    psy, lhsT=wo01_sb[:, dj * 128 : (dj + 1) * 128], rhs=ot01,
                    start=True, stop=False, skip_group_check=True,
                )
                nc.tensor.matmul(
                    psy, lhsT=wo2_sb[:, dj * 128 : (dj + 1) * 128], rhs=ot2,
                    start=False, stop=True, skip_group_check=True,
                )
                nc.vector.tensor_copy(y_acc[:, dj, :], psy)
                if dj == ND - 1:  # one batched DMA per q-block
                    nc.sync.dma_start(
                        out=y_d[:, qblk].rearrange("(j p) q -> p j q", p=128),
                        in_=y_acc,
                    )
            return thunk

        pending.extend(make_outproj(dj) for dj in range(ND))

    for thunk in pending:
        thunk()
    ctx.close()


def build():
    if "nc" in _CACHE:
        return _CACHE["nc"]
    nc = bacc.Bacc(
        "TRN2", target_bir_lowering=False, debug=False, num_devices=NCORES
    )
    with tile.TileContext(nc) as tc:
        _emit(tc)
    nc.compile()
    _CACHE["nc"] = nc
    return nc


def make_in_maps(x, w_qkv, w_out):
    import ml_dtypes

    bf16 = ml_dtypes.bfloat16
    x = np.asarray(x, dtype=np.float32)
    w_qkv = np.asarray(w_qkv, dtype=np.float32)
    w_out = np.asarray(w_out, dtype=np.float32)
    wq = w_qkv[0:D]        # [768, 768], rows = q features
    wk = w_qkv[D : 2 * D]
    wv = w_qkv[2 * D :]
    in_maps = []
    for c in range(NCORES):
        b, g = divmod(c, 4)
        hs = [3 * g + j for j in range(HL)]  # global head ids
        h0, h1, h2 = hs
        cols = []
        # e-tiles: [K0|K1] -> KA, [Q0|Q1] -> QB, [K2|Q2] -> C2, [Q2|K2] -> D2
        for pair in ((wk, h0), (wk, h1), (wq, h0), (wq, h1),
                     (wk, h2), (wq, h2), (wq, h2), (wk, h2)):
            w, h = pair
            cols.append(w[h * DK : (h + 1) * DK].T)  # [768, 64]
        wqkT = np.concatenate(cols, axis=1).astype(bf16)  # [768, 512]
        wvT = np.concatenate(
            [wv[h * DK : (h + 1) * DK].T for h in hs], axis=1
        ).astype(bf16)  # [768, 192]
        wo01T = np.concatenate(
            [w_out[:, h * DK : (h + 1) * DK].T for h in (h0, h1)], axis=0
        ).astype(bf16)  # [128, 768]
        wo2T = w_out[:, h2 * DK : (h2 + 1) * DK].T.astype(bf16)  # [64, 768]
        xT = np.ascontiguousarray(x[b].T).astype(bf16)  # [768, 4096]
        in_maps.append(
            {"xT": xT, "wqkT": wqkT, "wvT": wvT, "wo01T": wo01T, "wo2T": wo2T}
        )
    return in_maps


def run(inputs, trace=False):
    """Run on hardware; returns (y [B,T,D] fp32, BassKernelResults)."""
    nc = build()
    in_maps = make_in_maps(inputs["x"], inputs["w_qkv"], inputs["w_out"])
    br = run_bass_kernel_spmd(nc, in_maps, list(range(NCORES)), trace=trace)
    y = np.zeros((B, T, D), dtype=np.float32)
    for c in range(NCORES):
        b = c // 4
        y[b] += np.asarray(br.results[c]["yT"]).T
    return y, br


def kernel(x, w_qkv, w_out):
    y, _ = run({"x": x, "w_qkv": w_qkv, "w_out": w_out})
    return y


# revision 17
# speedup vs baseline: 1.0792x; 1.0792x over previous
"""Multi-head causal self-attention (B=2, T=4096, D=768, H=12) on 8 trn2 cores.

Sharding: core c -> batch b = c//4, heads 3*(c%4) .. 3*(c%4)+2.
qkv_proj column-parallel (each core computes Q/K/V only for its heads),
out_proj row-parallel (each core emits a partial y^T; host sums the 4
partials per batch).

v5 (bf16, one continuous exp pipeline): all matmul operands are bf16
(fp32 matmuls run at 1/4 PE rate; bf16 at full rate), accumulation
stays fp32 in PSUM.  x is pre-transposed on the host so the kernel
needs no PE transposes.  The ScalarE exp stream is the bottleneck
(~1us per [128,1024] score tile, ~215us total) and the whole kernel is
one software pipeline that keeps it saturated:

- attention work is a flat list of units (per q-block: nk k-tiles for
  row-paired heads 0/1, then nk/2 k-tile pairs for self-paired head 2);
  each unit's score matmuls are emitted one unit ahead of its exp, so
  the PE always runs one step ahead of ScalarE - including across
  head-pass and q-block boundaries.
- scores: heads 0/1 row-paired on opposite PE halves (concurrent via
  row groups) into a 2-bank PSUM tile, one exp covers both heads;
  head 2 self-pairs k-tile pairs via a swapped [Q2|K2] projection (D2).
- AV accumulates per head with a ones-row appended to V so softmax
  denominators fall out as row 64; per-head PSUM slots (tag "av",
  bufs=3) so head-2 never waits on the head-0/1 normalize.
- normalization: DVE approx-reciprocal (~18 bits, 5x faster than exact;
  input staged to SBUF - the custom op misreads PSUM) + gpsimd
  partition_broadcast + DVE multiply, all off the exp critical path.
- projections for t-block qb+1 and the out-projection of block qb-1
  are drip-fed between units (projections first - out-proj waits until
  the normalize chain has settled); y^T is staged in SBUF and written
  out in two DMAs per q-block.
"""

import sys

sys.path.insert(0, "/opt/trn_rl_repo")

import numpy as np
from contextlib import ExitStack

import concourse.bass as bass
import concourse.bacc as bacc
import concourse.tile as tile
import concourse.mybir as mybir
from concourse.bass_utils import run_bass_kernel_spmd

F32 = mybir.dt.float32
BF16 = mybir.dt.bfloat16
AF = mybir.ActivationFunctionType

B = 2
T = 4096
D = 768
H = 12
DK = 64
NCORES = 8
HL = 3  # heads per core
ND = D // 128  # 6 d-tiles
NKT = T // 128  # 32 k-tiles
NQB = T // 512  # 8 q-blocks

_CACHE = {}


def _emit(tc):
    nc = tc.nc
    xT_d = nc.dram_tensor("xT", [D, T], BF16, kind="ExternalInput").ap()
    wqk_d = nc.dram_tensor("wqkT", [D, 8 * DK], BF16, kind="ExternalInput").ap()
    wv_d = nc.dram_tensor("wvT", [D, HL * DK], BF16, kind="ExternalInput").ap()
    wo01_d = nc.dram_tensor("wo01T", [128, D], BF16, kind="ExternalInput").ap()
    wo2_d = nc.dram_tensor("wo2T", [DK, D], BF16, kind="ExternalInput").ap()
    y_d = nc.dram_tensor("yT", [D, T], F32, kind="ExternalOutput").ap()

    ctx = ExitStack()
    const = ctx.enter_context(tc.tile_pool(name="const", bufs=1))
    persist = ctx.enter_context(tc.tile_pool(name="persist", bufs=1))
    xtpool = ctx.enter_context(tc.tile_pool(name="xt", bufs=2))
    ptpool = ctx.enter_context(tc.tile_pool(name="pt", bufs=6))
    spool = ctx.enter_context(tc.tile_pool(name="sp", bufs=6))
    # PSUM (8 banks): psS tag "ps" 2x[128,1024] = 4 banks (score tiles,
    # double-buffered for the exp stream); psAV tag "av" 3x[65,512] = 3
    # banks (one AV accumulator per head); tag "x" 1x[128,512] = 1 bank
    # (drip-fed projection chunks + out-proj).
    psS = ctx.enter_context(tc.tile_pool(name="psS", bufs=2, space="PSUM"))
    psAV = ctx.enter_context(tc.tile_pool(name="psAV", bufs=1, space="PSUM"))

    # ---- weights ----
    wqk_sb = const.tile([128, ND, 8 * DK], BF16)
    nc.sync.dma_start(out=wqk_sb, in_=wqk_d.rearrange("(j p) e -> p j e", p=128))
    wv_sb = const.tile([128, ND, HL * DK], BF16)
    nc.sync.dma_start(out=wv_sb, in_=wv_d.rearrange("(j p) e -> p j e", p=128))
    wo01_sb = const.tile([128, D], BF16)
    nc.sync.dma_start(out=wo01_sb, in_=wo01_d)
    wo2_sb = const.tile([DK, D], BF16)
    nc.sync.dma_start(out=wo2_sb, in_=wo2_d)

    # warm the exp table set while weights stream in
    warm_in = const.tile([1, 16], F32)
    nc.vector.memset(warm_in, 0.0)
    warm_out = const.tile([1, 16], F32)
    nc.scalar.activation(warm_out, warm_in, AF.Exp, scale=0.125)

    # causal band masks, [k, q]-layout: mask[bp][k, q] = (q >= 128*bp + k).
    # bandA[bp]: same mask in both 512-halves (heads 0/1, same k-tile).
    # bandB[j]:  halves are bp=2j and bp=2j+1 (head 2, k-tile pair).
    def band_fill(m, half, bp):
        nc.gpsimd.affine_select(
            out=m[:, half * 512 : (half + 1) * 512],
            in_=m[:, half * 512 : (half + 1) * 512],
            compare_op=mybir.AluOpType.is_ge, fill=0.0,
            base=-128 * bp, pattern=[[1, 512]], channel_multiplier=-1,
        )

    bandA = []
    for bp in range(4):
        m = const.tile([128, 1024], BF16, name=f"bandA{bp}")
        nc.gpsimd.memset(m, 1.0)
        band_fill(m, 0, bp)
        band_fill(m, 1, bp)
        bandA.append(m)
    bandB = []
    for j in range(2):
        m = const.tile([128, 1024], BF16, name=f"bandB{j}")
        nc.gpsimd.memset(m, 1.0)
        band_fill(m, 0, 2 * j)
        band_fill(m, 1, 2 * j + 1)
        bandB.append(m)

    # ---- persistent activations ----
    # KA: [K^T_h0 ; K^T_h1], QB: [Q^T_h0 ; Q^T_h1] on partition halves
    KA = persist.tile([128, T], BF16, name="KA")
    QB = persist.tile([128, T], BF16, name="QB")
    C2 = persist.tile([128, T], BF16, name="C2")  # [K^T_h2 ; Q^T_h2]
    D2 = persist.tile([128, T], BF16, name="D2")  # [Q^T_h2 ; K^T_h2]
    # V natural [t, e] per k-tile with a ones col at e=64 -> softmax sums
    Vall = persist.tile([128, NKT, HL, DK + 1], BF16, name="Vall")
    nc.gpsimd.memset(Vall[:, :, :, DK : DK + 1], 1.0)
    ot01 = persist.tile([128, 512], BF16, name="ot01")  # heads 0/1 out^T
    ot2 = persist.tile([DK, 512], BF16, name="ot2")
    y_acc = persist.tile([128, ND, 512], F32, name="y_acc")  # out-proj staging

    qk_dest = [KA, QB, C2, D2]
    xts = {}

    def emit_xt_dma(tsb):
        tblk = slice(tsb * 512, (tsb + 1) * 512)
        xt_sb = xtpool.tile([128, ND, 512], BF16, name="xt_sb")
        nc.sync.dma_start(
            out=xt_sb, in_=xT_d[:, tblk].rearrange("(j p) t -> p j t", p=128)
        )
        xts[tsb] = xt_sb

    def proj_chunks(tsb, pool, tag):
        """PE chunk thunks for t-superblock tsb's projections (~0.6-1.3us
        of PE work each)."""
        tblk = slice(tsb * 512, (tsb + 1) * 512)

        def c_q(et):
            # et=3 projects [Q2|K2] (swapped weight columns) straight into
            # D2 - no partition-swap DMA needed
            def thunk():
                xt_sb = xts[tsb]
                ps_q = pool.tile([128, 512], F32, name="ps_q", tag=tag)
                for dj in range(ND):
                    nc.tensor.matmul(
                        ps_q,
                        lhsT=wqk_sb[:, dj, et * 128 : (et + 1) * 128],
                        rhs=xt_sb[:, dj, :],
                        start=(dj == 0), stop=(dj == ND - 1),
                    )
                nc.vector.tensor_copy(qk_dest[et][:, tblk], ps_q)
            return thunk

        def c_v(tt):
            def thunk():
                xt_sb = xts[tsb]
                kt = tsb * 4 + tt
                ps_v = pool.tile([128, HL * DK], F32, name="ps_v", tag=tag)
                for dj in range(ND):
                    nc.tensor.matmul(
                        ps_v,
                        lhsT=xt_sb[:, dj, tt * 128 : (tt + 1) * 128],
                        rhs=wv_sb[:, dj, :],
                        start=(dj == 0), stop=(dj == ND - 1),
                    )
                nc.vector.tensor_copy(
                    Vall[:, kt, :, 0:DK],
                    ps_v.rearrange("p (h e) -> p h e", h=HL),
                )
            return thunk

        return [c_q(0), c_q(1), c_q(2), c_q(3),
                c_v(0), c_v(1), c_v(2), c_v(3)]

    # phase A for t-block 0 runs upfront through the (currently idle)
    # wide psS pool so consecutive chunks double-buffer
    emit_xt_dma(0)
    if NQB > 1:
        emit_xt_dma(1)
    for thunk in proj_chunks(0, psS, "ps"):
        thunk()

    # ================= attention =================
    def normalize(av, dest):
        """dest[e, q] = av[e, q] / av[64, q] (softmax denominators)."""
        sums_sb = spool.tile([1, 512], F32, name="sums_sb")
        nc.vector.tensor_copy(sums_sb, av[DK : DK + 1, :])
        recip = spool.tile([1, 512], F32, name="recip")
        nc.vector.reciprocal_approx_fast(recip, sums_sb)
        recipb = spool.tile([DK, 512], F32, name="recipb")
        nc.gpsimd.partition_broadcast(recipb, recip, channels=DK)
        nc.vector.tensor_mul(dest, av[0:DK, :], recipb)

    def make_outproj(dj, qblk, tag):
        def thunk():
            pool = psAV if tag == "x" else psS
            psy = pool.tile([128, 512], F32, name="psy", tag=tag)
            nc.tensor.matmul(
                psy, lhsT=wo01_sb[:, dj * 128 : (dj + 1) * 128], rhs=ot01,
                start=True, stop=False, skip_group_check=True,
            )
            nc.tensor.matmul(
                psy, lhsT=wo2_sb[:, dj * 128 : (dj + 1) * 128], rhs=ot2,
                start=False, stop=True, skip_group_check=True,
            )
            nc.vector.tensor_copy(y_acc[:, dj, :], psy)
            if dj in (2, ND - 1):  # two DMAs per q-block overlap better
                half = slice(0, 3) if dj == 2 else slice(3, ND)
                rows = slice(half.start * 128, half.stop * 128)
                nc.sync.dma_start(
                    out=y_d[rows, qblk].rearrange("(j p) q -> p j q", p=128),
                    in_=y_acc[:, half, :],
                )
        return thunk

    # Flat unit list: each unit is (emit_scores, process).  Scores are
    # emitted one unit ahead so the PE always runs ahead of ScalarE.
    units = []
    for qb in range(NQB):
        nk = 4 * (qb + 1)
        qblk = slice(qb * 512, (qb + 1) * 512)
        state = {}

        def mk_scores01(kt, qblk=qblk):
            def emit():
                pss = psS.tile([128, 1024], F32, name="pss", tag="ps")
                kblk = slice(kt * 128, (kt + 1) * 128)
                nc.tensor.matmul(
                    pss[:, 0:512], lhsT=KA[0:64, kblk], rhs=QB[0:64, qblk],
                    start=True, stop=True,
                )
                nc.tensor.matmul(
                    pss[:, 512:1024], lhsT=KA[64:128, kblk],
                    rhs=QB[64:128, qblk], start=True, stop=True,
                )
                return pss
            return emit

        def mk_proc01(kt, qb=qb, nk=nk, state=state):
            def proc(pss):
                if kt == 0:
                    state["av"] = [
                        psAV.tile([DK + 1, 512], F32, name=f"av{h}",
                                  tag="av", bufs=3)
                        for h in (0, 1)
                    ]
                pt = ptpool.tile([128, 1024], BF16, name="pt")
                nc.scalar.activation(pt, pss, AF.Exp, scale=0.125)
                if kt >= 4 * qb:
                    nc.vector.tensor_mul(pt, pt, bandA[kt - 4 * qb])
                for h in (0, 1):
                    nc.tensor.matmul(
                        state["av"][h],
                        lhsT=Vall[:, kt, h, :],
                        rhs=pt[:, h * 512 : (h + 1) * 512],
                        start=(kt == 0), stop=(kt == nk - 1),
                    )
            return proc

        def mk_scores2(kp, qblk=qblk):
            def emit():
                pss = psS.tile([128, 1024], F32, name="pss2", tag="ps")
                b0 = slice((2 * kp) * 128, (2 * kp + 1) * 128)
                b1 = slice((2 * kp + 1) * 128, (2 * kp + 2) * 128)
                nc.tensor.matmul(
                    pss[:, 0:512], lhsT=C2[0:64, b0], rhs=D2[0:64, qblk],
                    start=True, stop=True,
                )
                nc.tensor.matmul(
                    pss[:, 512:1024], lhsT=D2[64:128, b1],
                    rhs=C2[64:128, qblk], start=True, stop=True,
                )
                return pss
            return emit

        def mk_proc2(kp, qb=qb, nk=nk, state=state):
            nkp = nk // 2

            def proc(pss):
                if kp == 0:
                    state["av2"] = psAV.tile(
                        [DK + 1, 512], F32, name="av2", tag="av", bufs=3
                    )
                pt2 = ptpool.tile([128, 1024], BF16, name="pt2")
                nc.scalar.activation(pt2, pss, AF.Exp, scale=0.125)
                if 2 * kp >= 4 * qb:
                    nc.vector.tensor_mul(pt2, pt2, bandB[kp - 2 * qb])
                nc.tensor.matmul(
                    state["av2"], lhsT=Vall[:, 2 * kp, 2, :],
                    rhs=pt2[:, 0:512], start=(kp == 0), stop=False,
                )
                nc.tensor.matmul(
                    state["av2"], lhsT=Vall[:, 2 * kp + 1, 2, :],
                    rhs=pt2[:, 512:1024], start=False, stop=(kp == nkp - 1),
                )
            return proc

        for kt in range(nk):
            units.append(
                dict(qb=qb, first=(kt == 0), last=False, qblk=qblk,
                     state=state, scores=mk_scores01(kt), proc=mk_proc01(kt))
            )
        for kp in range(nk // 2):
            units.append(
                dict(qb=qb, first=False, last=(kp == nk // 2 - 1), qblk=qblk,
                     state=state, scores=mk_scores2(kp), proc=mk_proc2(kp))
            )

    pend_chunks = []  # projections for t-block qb+1 (gate the next q-block)
    pend_out = []     # out-projection of q-block qb-1

    pss_next = units[0]["scores"]()
    unit_pos = 0
    for u_idx, u in enumerate(units):
        qb = u["qb"]
        if u["first"]:
            unit_pos = 0
            if qb + 2 < NQB:
                emit_xt_dma(qb + 2)
            if qb + 1 < NQB:
                pend_chunks.extend(proj_chunks(qb + 1, psAV, "x"))
        pss_cur = pss_next
        if u_idx + 1 < len(units):
            pss_next = units[u_idx + 1]["scores"]()
        # drip-feed: projections first; pop extra chunks if the remaining
        # units of this q-block could not absorb them all
        left = 6 * (qb + 1) - unit_pos
        popped = False
        while pend_chunks and (not popped or len(pend_chunks) >= left):
            pend_chunks.pop(0)()
            popped = True
        if not popped and pend_out:
            pend_out.pop(0)()
        unit_pos += 1
        u["proc"](pss_cur)
        if u["last"]:
            # out-proj of qb-1 must land before ot01/ot2 are rewritten
            while pend_out:
                pend_out.pop(0)()
            st = u["state"]
            normalize(st["av"][0], ot01[0:DK, :])
            ot1s = spool.tile([DK, 512], BF16, name="ot1s")
            normalize(st["av"][1], ot1s)
            nc.sync.dma_start(out=ot01[DK:128, :], in_=ot1s)
            normalize(st["av2"], ot2)
            tag = "ps" if qb == NQB - 1 else "x"
            pend_out.extend(
                make_outproj(dj, u["qblk"], tag) for dj in range(ND)
            )

    while pend_out:
        pend_out.pop(0)()
    ctx.close()


def build():
    if "nc" in _CACHE:
        return _CACHE["nc"]
    nc = bacc.Bacc(
        "TRN2", target_bir_lowering=False, debug=False, num_devices=NCORES
    )
    with tile.TileContext(nc) as tc:
        _emit(tc)
    nc.compile()
    _CACHE["nc"] = nc
    return nc


def make_in_maps(x, w_qkv, w_out):
    import ml_dtypes

    bf16 = ml_dtypes.bfloat16
    x = np.asarray(x, dtype=np.float32)
    w_qkv = np.asarray(w_qkv, dtype=np.float32)
    w_out = np.asarray(w_out, dtype=np.float32)
    wq = w_qkv[0:D]        # [768, 768], rows = q features
    wk = w_qkv[D : 2 * D]
    wv = w_qkv[2 * D :]
    in_maps = []
    for c in range(NCORES):
        b, g = divmod(c, 4)
        hs = [3 * g + j for j in range(HL)]  # global head ids
        h0, h1, h2 = hs
        cols = []
        # e-tiles: [K0|K1] -> KA, [Q0|Q1] -> QB, [K2|Q2] -> C2, [Q2|K2] -> D2
        for pair in ((wk, h0), (wk, h1), (wq, h0), (wq, h1),
                     (wk, h2), (wq, h2), (wq, h2), (wk, h2)):
            w, h = pair
            cols.append(w[h * DK : (h + 1) * DK].T)  # [768, 64]
        wqkT = np.concatenate(cols, axis=1).astype(bf16)  # [768, 512]
        wvT = np.concatenate(
            [wv[h * DK : (h + 1) * DK].T for h in hs], axis=1
        ).astype(bf16)  # [768, 192]
        wo01T = np.concatenate(
            [w_out[:, h * DK : (h + 1) * DK].T for h in (h0, h1)], axis=0
        ).astype(bf16)  # [128, 768]
        wo2T = w_out[:, h2 * DK : (h2 + 1) * DK].T.astype(bf16)  # [64, 768]
        xT = np.ascontiguousarray(x[b].T).astype(bf16)  # [768, 4096]
        in_maps.append(
            {"xT": xT, "wqkT": wqkT, "wvT": wvT, "wo01T": wo01T, "wo2T": wo2T}
        )
    return in_maps


def run(inputs, trace=False):
    """Run on hardware; returns (y [B,T,D] fp32, BassKernelResults)."""
    nc = build()
    in_maps = make_in_maps(inputs["x"], inputs["w_qkv"], inputs["w_out"])
    br = run_bass_kernel_spmd(nc, in_maps, list(range(NCORES)), trace=trace)
    y = np.zeros((B, T, D), dtype=np.float32)
    for c in range(NCORES):
        b = c // 4
        y[b] += np.asarray(br.results[c]["yT"]).T
    return y, br


def kernel(x, w_qkv, w_out):
    y, _ = run({"x": x, "w_qkv": w_qkv, "w_out": w_out})
    return y
